# revision 13
# baseline (speedup 1.0000x reference)
"""Trainium2 Bass kernel for GQA attention (nn_Attention_40364102648437).

Problem: B=2, S=2048, HIDDEN=896, 14 q heads / 2 kv heads, head_dim 64,
RoPE (theta 1e6), causal softmax, o-projection.

Sharding (8 cores, SPMD): core = b*4 + kv*2 + half.
Each core owns one batch b, one kv head, and 4 q-head slots (7 q heads per
kv group are split 4+3; the last slot of the second half is a duplicate
whose wo rows are zeroed so its contribution vanishes). Every core computes
a full [S, HIDDEN] partial output (its heads' contribution through wo);
the host sums the 4 partials per batch.

Engine budget (cost model): PE does all matmuls (~92us at 2.4GHz), the
Activation engine does ONLY the softmax exp (72 x [128,1024] ~= 75us), DVE
does RoPE/mask/bias/normalize element-wise work in bf16 (2-byte dtypes get
2-4x DVE rate), Pool (gpsimd) does PSUM->SBUF copies and the softmax
1/Z partition-broadcast, SP issues every DMA. The whole attention path is
bf16 (same PE rate as fp32r, half the DMA bytes, no fp32r-producer rule).

Softmax normalization: V tiles carry a ones column so PV accumulates the
denominator Z in o_ab row 64; 1/Z is computed by DVE reciprocal into a
[1,512] SBUF row, replicated across partitions with gpsimd
partition_broadcast (no DRAM bounce), and multiplied into the attention
output as it is copied to SBUF (bf16) for the o-projection.

Pipelining: PV trails scores/exp by one k-group; the o-projection of
superblock J is emitted after attention of J+1's first pair so the PE
never waits on the slot-b restack DMA; output rows DMA straight from an
SBUF staging tile.

Hardware constraints (from the previous session, kept intact):
  - concurrent row-group matmuls (partition bases 0/64) must write
    different PSUM banks -> s_ps puts slot a in bank 0, slot b in bank 1;
  - matmul start=True clears has_written for its PSUM region, so o_ab
    gets exactly one start/stop accumulation group;
  - engines cannot move data across partitions: RoPE's rotate-half swap,
    the k row duplication, and the slot-b restack use SBUF->SBUF DMA;
  - tensor_tensor may read only one input from PSUM.
"""

import numpy as np
import ml_dtypes

import concourse.bass as bass
import concourse.mybir as mybir
from concourse import bacc
from concourse.tile import TileContext
from concourse.masks import make_identity
from concourse.bass_utils import run_bass_kernel_spmd

F32 = mybir.dt.float32
BF16 = mybir.dt.bfloat16
BF = ml_dtypes.bfloat16

HIDDEN = 896
HEAD_DIM = 64
B = 2
S = 2048
ROPE_THETA = 1000000.0
NH7 = HIDDEN // 128  # 7 hidden tiles
NKB = S // 128       # 16 key blocks
NJ = S // 256        # 8 query superblocks (256 q positions each)


def build_program():
    nc = bacc.Bacc("TRN2", target_bir_lowering=False, debug=False, num_devices=8)

    # host-pre-tiled: row ss*128+p holds [t, n] -> hs[b][ss*512+n, t*128+p]
    hsT = nc.dram_tensor("hsT", [4 * 128, NH7 * 512], BF16, kind="ExternalInput")
    wq4 = nc.dram_tensor("wq4", [HIDDEN, 256], BF16, kind="ExternalInput")
    bq4 = nc.dram_tensor("bq4", [2, 128], F32, kind="ExternalInput")
    wkv = nc.dram_tensor("wkv", [HIDDEN, 128], BF16, kind="ExternalInput")
    bkv = nc.dram_tensor("bkv", [1, 128], F32, kind="ExternalInput")
    wo4 = nc.dram_tensor("wo4", [256, HIDDEN], BF16, kind="ExternalInput")
    cosd = nc.dram_tensor("cosd", [128, S], BF16, kind="ExternalInput")
    sind = nc.dram_tensor("sind", [128, S], BF16, kind="ExternalInput")
    maskD = nc.dram_tensor("maskD", [128, 1024], BF16, kind="ExternalInput")
    out_d = nc.dram_tensor("out", [S, HIDDEN], F32, kind="ExternalOutput")

    EXP = mybir.ActivationFunctionType.Exp

    with TileContext(nc) as tc:
        with (
            tc.tile_pool(name="const", bufs=1) as cpool,
            tc.tile_pool(name="big", bufs=1) as bigpool,
        ):
            # ---- constants, issued in first-use order (DMAs serialize on
            # the single HWDGE device at ~625ns each)
            wkv_sb = cpool.tile([128, NH7 * 128], BF16)
            nc.sync.dma_start(
                out=wkv_sb[:].rearrange("p (t f) -> p t f", t=NH7),
                in_=wkv.rearrange("(t p) f -> p t f", p=128),
            )
            bkv_sb = cpool.tile([128, 1], F32)
            nc.sync.dma_start(out=bkv_sb[:], in_=bkv.rearrange("a p -> p a"))
            wq_sb = cpool.tile([128, NH7 * 256], BF16)
            bq_sb = cpool.tile([128, 2], F32)
            cos_sb = cpool.tile([64, S], BF16)
            sin_sb = cpool.tile([64, S], BF16)
            wo_sb = cpool.tile([128, 2 * HIDDEN], BF16)
            mask_sb = cpool.tile([128, 1024], BF16)
            identb = cpool.tile([128, 128], BF16)
            make_identity(nc, identb[:])

            def load_consts():
                nc.sync.dma_start(out=cos_sb[:], in_=cosd[0:64, :])
                nc.sync.dma_start(out=sin_sb[:], in_=sind[0:64, :])
                nc.sync.dma_start(
                    out=wq_sb[:].rearrange("p (t f) -> p t f", t=NH7),
                    in_=wq4.rearrange("(t p) f -> p t f", p=128),
                )
                nc.sync.dma_start(out=bq_sb[:], in_=bq4.rearrange("a p -> p a"))
                nc.sync.dma_start(
                    out=wo_sb[:].rearrange("p (t f) -> p t f", t=2),
                    in_=wo4.rearrange("(t p) f -> p t f", p=128),
                )
                nc.sync.dma_start(out=mask_sb[:], in_=maskD[:])

            # ---- persistent activations (all bf16)
            kvT = bigpool.tile([128, S], BF16)   # rows 0-63 k, 64-127 vT
            kdr = bigpool.tile([128, S], BF16)   # rope'd k, duplicated halves
            qA = bigpool.tile([128, S], BF16)
            qB = bigpool.tile([128, S], BF16)
            qAr = bigpool.tile([128, S], BF16)
            qBr = bigpool.tile([128, S], BF16)
            v_sb = bigpool.tile([128, NKB * 65], BF16)
            aoT0 = bigpool.tile([128, S], BF16)
            aoT1 = bigpool.tile([128, S], BF16)
            stg0 = bigpool.tile([64, S], BF16)
            stg1 = bigpool.tile([64, S], BF16)

            # ================= phase A: projections =================
            with (
                tc.tile_pool(name="hst", bufs=4) as hpool,
                tc.tile_pool(name="pps", bufs=2, space="PSUM") as ppool,
                tc.tile_pool(name="swp", bufs=3) as swpool,
                tc.tile_pool(name="vtr", bufs=2, space="PSUM") as vpool,
            ):
                hs_tiles = []
                for ss in range(4):
                    ssl = slice(ss * 512, (ss + 1) * 512)
                    hs_t = hpool.tile([128, NH7 * 512], BF16)
                    hs_tiles.append(hs_t)
                    if ss == 0:
                        # split so the first kv matmuls start sooner
                        nc.sync.dma_start(
                            out=hs_t[:, 0 : 3 * 512],
                            in_=hsT[0:128, 0 : 3 * 512],
                        )
                        nc.sync.dma_start(
                            out=hs_t[:, 3 * 512 :],
                            in_=hsT[0:128, 3 * 512 :],
                        )
                    else:
                        nc.sync.dma_start(
                            out=hs_t[:], in_=hsT[ss * 128 : (ss + 1) * 128, :]
                        )
                    if ss == 3:
                        # remaining constants queue behind the hs tiles
                        load_consts()
                    kv_ps = ppool.tile([128, 512], F32)
                    for h in range(NH7):
                        nc.tensor.matmul(
                            kv_ps[:],
                            wkv_sb[:, h * 128 : (h + 1) * 128],
                            hs_t[:, h * 512 : (h + 1) * 512],
                            start=(h == 0),
                            stop=(h == NH7 - 1),
                        )
                    nc.vector.tensor_scalar_add(kvT[:, ssl], kv_ps[:], bkv_sb[:, 0:1])

                # ---- RoPE on k (rows 0-63 of kvT), then duplicate halves.
                # Swaps are SBUF->SBUF DMAs on the Pool queue (SWDGE path,
                # no HWDGE contention with the hs/weight loads); the PE
                # proceeds straight into the V transposes + q projections,
                # hiding the rope latency.
                ktw = swpool.tile([64, S], BF16)
                nc.gpsimd.dma_start(out=ktw[0:32, :], in_=kvT[32:64, :])
                nc.gpsimd.dma_start(out=ktw[32:64, :], in_=kvT[0:32, :])
                nc.vector.tensor_mul(ktw[:], ktw[:], sin_sb[:])
                nc.vector.tensor_mul(kvT[0:64, :], kvT[0:64, :], cos_sb[:])
                nc.vector.tensor_add(kdr[0:64, :], kvT[0:64, :], ktw[:])
                nc.gpsimd.dma_start(out=kdr[64:128, :], in_=kdr[0:64, :])

                # ---- v natural layout [k_pos, 64] + ones column (col 64)
                for kb in range(NKB):
                    vt_ps = vpool.tile([128, 64], BF16)
                    nc.tensor.transpose(
                        vt_ps[:],
                        kvT[64:128, kb * 128 : (kb + 1) * 128],
                        identb[64:128, 64:128],
                    )
                    nc.gpsimd.tensor_copy(v_sb[:, kb * 65 : kb * 65 + 64], vt_ps[:])
                ones_ap = v_sb[:, 64 : NKB * 65 : 65]
                nc.gpsimd.memset(ones_ap, 1.0)

                # ---- q projections (hs tiles stay resident) + RoPE
                for ss in range(4):
                    ssl = slice(ss * 512, (ss + 1) * 512)
                    hs_t = hs_tiles[ss]
                    for ft in range(2):
                        q_ps = ppool.tile([128, 512], F32)
                        for h in range(NH7):
                            nc.tensor.matmul(
                                q_ps[:],
                                wq_sb[:, h * 256 + ft * 128 : h * 256 + (ft + 1) * 128],
                                hs_t[:, h * 512 : (h + 1) * 512],
                                start=(h == 0),
                                stop=(h == NH7 - 1),
                            )
                        qt = (qA, qB)[ft]
                        nc.vector.tensor_scalar_add(
                            qt[:, ssl], q_ps[:], bq_sb[:, ft : ft + 1]
                        )
                for t, tr in ((qA, qAr), (qB, qBr)):
                    tsw = swpool.tile([128, S], BF16)
                    for dst, src in ((0, 32), (32, 0), (64, 96), (96, 64)):
                        nc.gpsimd.dma_start(
                            out=tsw[dst : dst + 32, :], in_=t[src : src + 32, :]
                        )
                    for hf in range(2):
                        hsl = slice(hf * 64, (hf + 1) * 64)
                        nc.vector.tensor_mul(tsw[hsl, :], tsw[hsl, :], sin_sb[:])
                        nc.vector.tensor_mul(t[hsl, :], t[hsl, :], cos_sb[:])
                        nc.vector.tensor_add(tr[hsl, :], t[hsl, :], tsw[hsl, :])

            # ================= phase B: attention + o-projection =================
            with (
                tc.tile_pool(name="sps", bufs=2, space="PSUM") as spool,
                tc.tile_pool(name="ops", bufs=2, space="PSUM") as opool,
                tc.tile_pool(name="fps", bufs=2, space="PSUM") as fpool,
                tc.tile_pool(name="esb", bufs=4) as epool,
                tc.tile_pool(name="rzs", bufs=2) as rzpool,
                tc.tile_pool(name="bcs", bufs=2) as bcpool,
                tc.tile_pool(name="osb", bufs=3) as obpool,
            ):
                def emit_oproj(J):
                    for qb in (2 * J, 2 * J + 1):
                        ob = obpool.tile([128, HIDDEN], F32)
                        for half in range(2):
                            hsl = slice(half * 448, (half + 1) * 448)
                            f_ps = fpool.tile([128, 448], F32)
                            for ft in range(2):
                                aoTt = (aoT0, aoT1)[ft]
                                nc.tensor.matmul(
                                    f_ps[:],
                                    aoTt[:, qb * 128 : (qb + 1) * 128],
                                    wo_sb[:, ft * HIDDEN + half * 448 :
                                          ft * HIDDEN + (half + 1) * 448],
                                    start=(ft == 0),
                                    stop=(ft == 1),
                                )
                            nc.gpsimd.tensor_copy(ob[:, hsl], f_ps[:])
                        nc.sync.dma_start(
                            out=out_d[qb * 128 : (qb + 1) * 128, :], in_=ob[:]
                        )

                for J in range(NJ):
                    for pair in range(2):
                        qt = (qAr, qBr)[pair]
                        aoT = (aoT0, aoT1)[pair]
                        stg = (stg0, stg1)[pair]
                        qsl = slice(J * 256, (J + 1) * 256)
                        o_ab = opool.tile([65, 512], F32)
                        pend = None  # software pipeline: PV trails S^T/exp by 1
                        # diagonal group first: its exp -> mask -> PV chain
                        # pipelines like any other group instead of stalling
                        # the PE at the end of the pair
                        gorder = [J] + list(range(J))
                        for gi, g in enumerate(gorder):
                            s_ps = spool.tile([128, 1024], F32)
                            for i, kb in enumerate((2 * g, 2 * g + 1)):
                                for half in range(2):
                                    # concurrent row-group pair must write
                                    # different PSUM banks: slot a bank 0,
                                    # slot b bank 1
                                    seg = half * 512 + i * 256
                                    nc.tensor.matmul(
                                        s_ps[:, seg : seg + 256],
                                        kdr[half * 64 : (half + 1) * 64,
                                            kb * 128 : (kb + 1) * 128],
                                        qt[half * 64 : (half + 1) * 64, qsl],
                                        start=True,
                                        stop=True,
                                    )
                            e_sb = epool.tile([128, 1024], BF16)
                            nc.scalar.activation(
                                e_sb[:], s_ps[:], EXP, bias=0.0, scale=0.125
                            )
                            if g == J:
                                # multiplicative 0/1 causal mask after exp
                                # (bf16, all-SBUF -> 4x DVE rate)
                                nc.vector.tensor_mul(e_sb[:], e_sb[:], mask_sb[:])
                            if pend is not None:
                                _emit_pv(nc, o_ab, v_sb, *pend)
                            pend = (e_sb, g, gi == 0, gi == J)
                        _emit_pv(nc, o_ab, v_sb, *pend)

                        # normalize: 1/Z from o_ab row 64, replicated across
                        # partitions on the Pool engine, multiplied in as the
                        # attention output is copied to SBUF (bf16)
                        rz = rzpool.tile([1, 512], F32)
                        nc.vector.reciprocal(rz[:], o_ab[64:65, :])
                        bc = bcpool.tile([64, 512], F32)
                        nc.gpsimd.partition_broadcast(bc[:], rz[0:1, :])
                        nc.vector.tensor_mul(
                            aoT[0:64, qsl], o_ab[0:64, 0:256], bc[:, 0:256]
                        )
                        nc.vector.tensor_mul(
                            stg[0:64, qsl], o_ab[0:64, 256:512], bc[:, 256:512]
                        )
                        # restack slot-b rows into partitions 64..127
                        nc.sync.dma_start(out=aoT[64:128, qsl], in_=stg[0:64, qsl])
                    # o-projection deferred one J so the PE never waits on
                    # the restack DMA chain
                    if J > 0:
                        emit_oproj(J - 1)
                emit_oproj(NJ - 1)

    nc.compile()
    return nc


def _emit_pv(nc, o_ab, v_sb, e_sb, g, first, last):
    """PV accumulation for one exp'd group (k-blocks 2g, 2g+1)."""
    for i, kb in enumerate((2 * g, 2 * g + 1)):
        for sl in range(2):
            seg = sl * 512 + i * 256
            # one accumulation group for the whole o_ab tile: start=True
            # clears has_written for the entire PSUM bank, so only the very
            # first matmul may set it
            nc.tensor.matmul(
                o_ab[:, sl * 256 : (sl + 1) * 256],
                v_sb[:, kb * 65 : (kb + 1) * 65],
                e_sb[:, seg : seg + 256],
                start=(first and i == 0 and sl == 0),
                stop=(last and i == 1 and sl == 1),
                skip_group_check=True,
            )


def _rope_tables():
    inv_freq = 1.0 / (
        ROPE_THETA ** (np.arange(0, HEAD_DIM, 2, dtype=np.float32) / HEAD_DIM)
    )
    t = np.arange(S, dtype=np.float32)
    freqs = np.outer(t, inv_freq)  # [S, 32]
    emb = np.concatenate([freqs, freqs], axis=-1)  # [S, 64]
    cosT = np.cos(emb).T.astype(np.float32)  # [64, S]
    sinT = np.sin(emb).T.astype(np.float32)
    sinmod = sinT.copy()
    sinmod[0:32] = -sinmod[0:32]
    cosd = np.concatenate([cosT, cosT], axis=0)  # [128, S]
    sind = np.concatenate([sinmod, sinmod], axis=0)
    return np.ascontiguousarray(cosd.astype(BF)), np.ascontiguousarray(
        sind.astype(BF)
    )


def _masks():
    kp = np.arange(128)[:, None]
    qp = np.arange(128)[None, :]
    tri = np.where(kp <= qp, 1.0, 0.0).astype(np.float32)  # [128,128]
    ones = np.ones((128, 128), np.float32)
    zeros = np.zeros((128, 128), np.float32)
    mask0 = np.concatenate([tri, ones], axis=1)   # kb 2J vs [2J, 2J+1]
    mask1 = np.concatenate([zeros, tri], axis=1)  # kb 2J+1 vs [2J, 2J+1]
    return np.ascontiguousarray(
        np.concatenate([mask0, mask1, mask0, mask1], axis=1).astype(BF)
    )  # [128, 1024]


def _tile_hsT(hsT):
    """[896, 2048] -> [512, 3584]: row ss*128+p = concat over t of
    hsT[t*128+p, ss*512:(ss+1)*512], matching the SBUF projection layout."""
    out = np.empty((4 * 128, NH7 * 512), BF)
    for ss in range(4):
        blk = hsT[:, ss * 512 : (ss + 1) * 512].reshape(NH7, 128, 512)
        out[ss * 128 : (ss + 1) * 128, :] = (
            blk.transpose(1, 0, 2).reshape(128, NH7 * 512).astype(BF)
        )
    return np.ascontiguousarray(out)


_CONST_CACHE = None


def make_in_maps(hidden_states, wq, bq, wk, bk, wv, bv, wo):
    global _CONST_CACHE
    if _CONST_CACHE is None:
        cosd, sind = _rope_tables()
        _CONST_CACHE = (cosd, sind, _masks())
    cosd, sind, maskD = _CONST_CACHE
    # the tiled hidden states are shared by the 4 cores of a batch
    hs_tiled = [_tile_hsT(hidden_states[b].T) for b in range(B)]
    in_maps = []
    for core in range(8):
        b, kv, half = core // 4, (core % 4) // 2, core % 2
        if half == 0:
            slots = [kv * 7 + 0, kv * 7 + 1, kv * 7 + 2, kv * 7 + 3]
            dup = []
        else:
            slots = [kv * 7 + 4, kv * 7 + 5, kv * 7 + 6, kv * 7 + 3]
            dup = [3]
        cols = np.concatenate([np.arange(h * 64, (h + 1) * 64) for h in slots])
        wq4 = np.ascontiguousarray(wq[:, cols].astype(BF))
        bq4 = np.ascontiguousarray(bq[cols].reshape(2, 128))
        wkv = np.ascontiguousarray(
            np.concatenate(
                [wk[:, kv * 64 : (kv + 1) * 64], wv[:, kv * 64 : (kv + 1) * 64]],
                axis=1,
            ).astype(BF)
        )
        bkv = np.ascontiguousarray(
            np.concatenate(
                [bk[kv * 64 : (kv + 1) * 64], bv[kv * 64 : (kv + 1) * 64]]
            ).reshape(1, 128)
        )
        wo4 = wo[cols, :].copy()
        for d in dup:
            wo4[d * 64 : (d + 1) * 64, :] = 0.0
        in_maps.append(
            {
                "hsT": hs_tiled[b],
                "wq4": wq4,
                "bq4": bq4,
                "wkv": wkv,
                "bkv": bkv,
                "wo4": np.ascontiguousarray(wo4.astype(BF)),
                "cosd": cosd,
                "sind": sind,
                "maskD": maskD,
            }
        )
    return in_maps


_NC_CACHE = None


def _get_program():
    global _NC_CACHE
    if _NC_CACHE is None:
        _NC_CACHE = build_program()
    return _NC_CACHE


def kernel(hidden_states, wq, bq, wk, bk, wv, bv, wo):
    hidden_states = np.asarray(hidden_states, np.float32)
    wq = np.asarray(wq, np.float32)
    bq = np.asarray(bq, np.float32)
    wk = np.asarray(wk, np.float32)
    bk = np.asarray(bk, np.float32)
    wv = np.asarray(wv, np.float32)
    bv = np.asarray(bv, np.float32)
    wo = np.asarray(wo, np.float32)

    nc = _get_program()
    in_maps = make_in_maps(hidden_states, wq, bq, wk, bk, wv, bv, wo)
    res = run_bass_kernel_spmd(nc, in_maps, list(range(8)))
    out = np.zeros((B, S, HIDDEN), np.float32)
    for core in range(8):
        out[core // 4] += res.results[core]["out"]
    return out


# revision 25
# speedup vs baseline: 1.0189x; 1.0189x over previous
"""Trainium2 Bass kernel for GQA attention (nn_Attention_40364102648437).

Problem: B=2, S=2048, HIDDEN=896, 14 q heads / 2 kv heads, head_dim 64,
RoPE (theta 1e6), causal softmax, o-projection.

Sharding (8 cores, SPMD): core = b*4 + kv*2 + half.
Each core owns one batch b, one kv head, and 4 q-head slots (7 q heads per
kv group are split 4+3; the last slot of the second half is a duplicate
whose wo rows are zeroed so its contribution vanishes). Every core computes
a full [S, HIDDEN] partial output (its heads' contribution through wo);
the host sums the 4 partials per batch.

Engine budget (cost model): PE does all matmuls (~92us at 2.4GHz), the
Activation engine does ONLY the softmax exp (72 x [128,1024] ~= 75us), DVE
does RoPE/mask/bias/normalize element-wise work in bf16 (2-byte dtypes get
2-4x DVE rate), Pool (gpsimd) does PSUM->SBUF copies and the softmax
1/Z partition-broadcast, SP issues every DMA. The whole attention path is
bf16 (same PE rate as fp32r, half the DMA bytes, no fp32r-producer rule).

Softmax normalization: V tiles carry a ones column so PV accumulates the
denominator Z in o_ab row 64; 1/Z is computed by DVE reciprocal into a
[1,512] SBUF row, replicated across partitions with gpsimd
partition_broadcast (no DRAM bounce), and multiplied into the attention
output as it is copied to SBUF (bf16) for the o-projection.

Pipelining: PV trails scores/exp by one k-group; the o-projection of
superblock J is emitted after attention of J+1's first pair so the PE
never waits on the slot-b restack DMA; output rows DMA straight from an
SBUF staging tile.

Hardware constraints (from the previous session, kept intact):
  - concurrent row-group matmuls (partition bases 0/64) must write
    different PSUM banks -> s_ps puts slot a in bank 0, slot b in bank 1;
  - matmul start=True clears has_written for its PSUM region, so o_ab
    gets exactly one start/stop accumulation group;
  - engines cannot move data across partitions: RoPE's rotate-half swap,
    the k row duplication, and the slot-b restack use SBUF->SBUF DMA;
  - tensor_tensor may read only one input from PSUM.
"""

import numpy as np
import ml_dtypes

import concourse.bass as bass
import concourse.mybir as mybir
from concourse import bacc
from concourse.tile import TileContext
from concourse.masks import make_identity
from concourse.bass_utils import run_bass_kernel_spmd

F32 = mybir.dt.float32
BF16 = mybir.dt.bfloat16
BF = ml_dtypes.bfloat16

HIDDEN = 896
HEAD_DIM = 64
B = 2
S = 2048
ROPE_THETA = 1000000.0
NH7 = HIDDEN // 128  # 7 hidden tiles
NKB = S // 128       # 16 key blocks
NJ = S // 256        # 8 query superblocks (256 q positions each)


def build_program():
    nc = bacc.Bacc("TRN2", target_bir_lowering=False, debug=False, num_devices=8)

    # host-pre-tiled: row ss*128+p holds [t, n] -> hs[b][ss*512+n, t*128+p]
    hsT = nc.dram_tensor("hsT", [4 * 128, NH7 * 512], BF16, kind="ExternalInput")
    wq4 = nc.dram_tensor("wq4", [HIDDEN, 256], BF16, kind="ExternalInput")
    bq4 = nc.dram_tensor("bq4", [2, 128], F32, kind="ExternalInput")
    wkv = nc.dram_tensor("wkv", [HIDDEN, 128], BF16, kind="ExternalInput")
    bkv = nc.dram_tensor("bkv", [1, 128], F32, kind="ExternalInput")
    wo4 = nc.dram_tensor("wo4", [256, HIDDEN], BF16, kind="ExternalInput")
    cosd = nc.dram_tensor("cosd", [64, S], BF16, kind="ExternalInput")
    sind = nc.dram_tensor("sind", [64, S], BF16, kind="ExternalInput")
    maskD = nc.dram_tensor("maskD", [128, 1024], BF16, kind="ExternalInput")
    # rotate-half permutations as matmul weights: permQ does the half-swap
    # within each 64-row slot; permKs/permKc (cols 128:256 / 256:384, rows
    # 0:64) swap and duplicate k into both partition halves
    permD = nc.dram_tensor("permD", [128, 384], BF16, kind="ExternalInput")
    out_d = nc.dram_tensor("out", [S, HIDDEN], F32, kind="ExternalOutput")

    EXP = mybir.ActivationFunctionType.Exp

    with TileContext(nc) as tc:
        with (
            tc.tile_pool(name="const", bufs=1) as cpool,
            tc.tile_pool(name="big", bufs=1) as bigpool,
        ):
            # ---- constants, issued in first-use order (DMAs serialize on
            # the single HWDGE device at ~625ns each)
            wkv_sb = cpool.tile([128, NH7 * 128], BF16)
            nc.sync.dma_start(
                out=wkv_sb[:].rearrange("p (t f) -> p t f", t=NH7),
                in_=wkv.rearrange("(t p) f -> p t f", p=128),
            )
            bkv_sb = cpool.tile([128, 1], F32)
            nc.sync.dma_start(out=bkv_sb[:], in_=bkv.rearrange("a p -> p a"))
            perm_sb = cpool.tile([128, 384], BF16)
            nc.sync.dma_start(out=perm_sb[:], in_=permD[:])
            wq_sb = cpool.tile([128, NH7 * 256], BF16)
            bq_sb = cpool.tile([128, 2], F32)
            cos_sb = cpool.tile([64, S], BF16)
            sin_sb = cpool.tile([64, S], BF16)
            wo_sb = cpool.tile([128, 2 * HIDDEN], BF16)
            mask_sb = cpool.tile([128, 1024], BF16)
            identb = cpool.tile([128, 128], BF16)
            make_identity(nc, identb[:])
            # force the Exp activation table load off the critical path
            warm = cpool.tile([1, 8], F32)
            nc.vector.memset(warm[:], 0.0)
            nc.scalar.activation(
                warm[:], warm[:], mybir.ActivationFunctionType.Exp,
                bias=0.0, scale=1.0,
            )

            def load_consts():
                nc.sync.dma_start(out=cos_sb[:], in_=cosd[:])
                nc.sync.dma_start(out=sin_sb[:], in_=sind[:])
                nc.sync.dma_start(
                    out=wq_sb[:].rearrange("p (t f) -> p t f", t=NH7),
                    in_=wq4.rearrange("(t p) f -> p t f", p=128),
                )
                nc.sync.dma_start(out=bq_sb[:], in_=bq4.rearrange("a p -> p a"))
                nc.sync.dma_start(
                    out=wo_sb[:].rearrange("p (t f) -> p t f", t=2),
                    in_=wo4.rearrange("(t p) f -> p t f", p=128),
                )
                nc.sync.dma_start(out=mask_sb[:], in_=maskD[:])

            # ---- persistent activations (all bf16)
            kvT = bigpool.tile([128, S], BF16)   # rows 0-63 k, 64-127 vT
            kdr = bigpool.tile([128, S], BF16)   # rope'd k, duplicated halves
            qA = bigpool.tile([128, S], BF16)
            qB = bigpool.tile([128, S], BF16)
            qAr = bigpool.tile([128, S], BF16)
            qBr = bigpool.tile([128, S], BF16)
            v_sb = bigpool.tile([128, NKB * 65], BF16)
            aoT0 = bigpool.tile([128, S], BF16)
            aoT1 = bigpool.tile([128, S], BF16)
            stg0 = bigpool.tile([64, S], BF16)
            stg1 = bigpool.tile([64, S], BF16)

            # ================= phase A: projections =================
            with (
                tc.tile_pool(name="hst", bufs=4) as hpool,
                tc.tile_pool(name="pps", bufs=3, space="PSUM") as ppool,
                tc.tile_pool(name="swp", bufs=3) as swpool,
                tc.tile_pool(name="vtr", bufs=2, space="PSUM") as vpool,
            ):
                hs_tiles = []
                for ss in range(4):
                    ssl = slice(ss * 512, (ss + 1) * 512)
                    hs_t = hpool.tile([128, NH7 * 512], BF16)
                    hs_tiles.append(hs_t)
                    if ss == 0:
                        # split so the first kv matmuls start sooner
                        nc.sync.dma_start(
                            out=hs_t[:, 0 : 3 * 512],
                            in_=hsT[0:128, 0 : 3 * 512],
                        )
                        nc.sync.dma_start(
                            out=hs_t[:, 3 * 512 :],
                            in_=hsT[0:128, 3 * 512 :],
                        )
                    else:
                        nc.sync.dma_start(
                            out=hs_t[:], in_=hsT[ss * 128 : (ss + 1) * 128, :]
                        )
                    if ss == 3:
                        # remaining constants queue behind the hs tiles
                        load_consts()
                    kv_ps = ppool.tile([128, 512], F32, tag="pps")
                    for h in range(NH7):
                        nc.tensor.matmul(
                            kv_ps[:],
                            wkv_sb[:, h * 128 : (h + 1) * 128],
                            hs_t[:, h * 512 : (h + 1) * 512],
                            start=(h == 0),
                            stop=(h == NH7 - 1),
                        )
                    nc.vector.tensor_scalar_add(kvT[:, ssl], kv_ps[:], bkv_sb[:, 0:1])

                # ---- RoPE on k (rows 0-63 of kvT): rotate-half runs on the
                # PE as a permutation matmul (no partition-moving DMA), and
                # permKs/permKc simultaneously duplicate the rope'd k into
                # both partition halves for the row-paired score matmuls.
                ks = swpool.tile([64, S], BF16)
                nc.vector.tensor_mul(ks[:], kvT[0:64, :], sin_sb[:])
                nc.vector.tensor_mul(kvT[0:64, :], kvT[0:64, :], cos_sb[:])
                for c in range(4):
                    csl = slice(c * 512, (c + 1) * 512)
                    r_ps = ppool.tile([128, 512], F32, tag="pps")
                    nc.tensor.matmul(
                        r_ps[:], perm_sb[0:64, 128:256], ks[:, csl],
                        start=True, stop=False,
                    )
                    nc.tensor.matmul(
                        r_ps[:], perm_sb[0:64, 256:384], kvT[0:64, csl],
                        start=False, stop=True,
                    )
                    nc.gpsimd.tensor_copy(kdr[:, csl], r_ps[:])

                # ---- v natural layout [k_pos, 64] + ones column (col 64)
                for kb in range(NKB):
                    vt_ps = vpool.tile([128, 64], BF16)
                    nc.tensor.transpose(
                        vt_ps[:],
                        kvT[64:128, kb * 128 : (kb + 1) * 128],
                        identb[64:128, 64:128],
                    )
                    nc.gpsimd.tensor_copy(v_sb[:, kb * 65 : kb * 65 + 64], vt_ps[:])
                ones_ap = v_sb[:, 64 : NKB * 65 : 65]
                nc.gpsimd.memset(ones_ap, 1.0)

                # ---- q projections (hs tiles stay resident) + RoPE
                for ss in range(4):
                    ssl = slice(ss * 512, (ss + 1) * 512)
                    hs_t = hs_tiles[ss]
                    for ft in range(2):
                        q_ps = ppool.tile([128, 512], F32, tag="pps")
                        for h in range(NH7):
                            nc.tensor.matmul(
                                q_ps[:],
                                wq_sb[:, h * 256 + ft * 128 : h * 256 + (ft + 1) * 128],
                                hs_t[:, h * 512 : (h + 1) * 512],
                                start=(h == 0),
                                stop=(h == NH7 - 1),
                            )
                        qt = (qA, qB)[ft]
                        nc.vector.tensor_scalar_add(
                            qt[:, ssl], q_ps[:], bq_sb[:, ft : ft + 1]
                        )
                for t, tr in ((qA, qAr), (qB, qBr)):
                    qs = swpool.tile([128, S], BF16)
                    for hf in range(2):
                        hsl = slice(hf * 64, (hf + 1) * 64)
                        nc.vector.tensor_mul(qs[hsl, :], t[hsl, :], sin_sb[:])
                        nc.vector.tensor_mul(t[hsl, :], t[hsl, :], cos_sb[:])
                    for c in range(4):
                        csl = slice(c * 512, (c + 1) * 512)
                        r_ps = ppool.tile([128, 512], F32, tag="pps")
                        nc.tensor.matmul(
                            r_ps[:], perm_sb[:, 0:128], qs[:, csl],
                            start=True, stop=False,
                        )
                        nc.tensor.matmul(
                            r_ps[:], identb[:], t[:, csl],
                            start=False, stop=True,
                        )
                        nc.gpsimd.tensor_copy(tr[:, csl], r_ps[:])

            # ================= phase B: attention + o-projection =================
            with (
                tc.tile_pool(name="sps", bufs=2, space="PSUM") as spool,
                tc.tile_pool(name="ops", bufs=2, space="PSUM") as opool,
                tc.tile_pool(name="fps", bufs=2, space="PSUM") as fpool,
                tc.tile_pool(name="esb", bufs=4) as epool,
                tc.tile_pool(name="rzs", bufs=2) as rzpool,
                tc.tile_pool(name="bcs", bufs=2) as bcpool,
                tc.tile_pool(name="osb", bufs=3) as obpool,
            ):
                def emit_oproj(J):
                    for qb in (2 * J, 2 * J + 1):
                        ob = obpool.tile([128, HIDDEN], F32)
                        for half in range(2):
                            hsl = slice(half * 448, (half + 1) * 448)
                            f_ps = fpool.tile([128, 448], F32)
                            for ft in range(2):
                                aoTt = (aoT0, aoT1)[ft]
                                nc.tensor.matmul(
                                    f_ps[:],
                                    aoTt[:, qb * 128 : (qb + 1) * 128],
                                    wo_sb[:, ft * HIDDEN + half * 448 :
                                          ft * HIDDEN + (half + 1) * 448],
                                    start=(ft == 0),
                                    stop=(ft == 1),
                                )
                            nc.gpsimd.tensor_copy(ob[:, hsl], f_ps[:])
                        nc.sync.dma_start(
                            out=out_d[qb * 128 : (qb + 1) * 128, :], in_=ob[:]
                        )

                for J in range(NJ):
                    for pair in range(2):
                        qt = (qAr, qBr)[pair]
                        aoT = (aoT0, aoT1)[pair]
                        stg = (stg0, stg1)[pair]
                        qsl = slice(J * 256, (J + 1) * 256)
                        o_ab = opool.tile([65, 512], F32)
                        pend = None  # software pipeline: PV trails S^T/exp by 1
                        # diagonal group first: its exp -> mask -> PV chain
                        # pipelines like any other group instead of stalling
                        # the PE at the end of the pair
                        gorder = [J] + list(range(J))
                        for gi, g in enumerate(gorder):
                            s_ps = spool.tile([128, 1024], F32)
                            for i, kb in enumerate((2 * g, 2 * g + 1)):
                                for half in range(2):
                                    # concurrent row-group pair must write
                                    # different PSUM banks: slot a bank 0,
                                    # slot b bank 1
                                    seg = half * 512 + i * 256
                                    nc.tensor.matmul(
                                        s_ps[:, seg : seg + 256],
                                        kdr[half * 64 : (half + 1) * 64,
                                            kb * 128 : (kb + 1) * 128],
                                        qt[half * 64 : (half + 1) * 64, qsl],
                                        start=True,
                                        stop=True,
                                    )
                            e_sb = epool.tile([128, 1024], BF16)
                            nc.scalar.activation(
                                e_sb[:], s_ps[:], EXP, bias=0.0, scale=0.125
                            )
                            if g == J:
                                # multiplicative 0/1 causal mask after exp
                                # (bf16, all-SBUF -> 4x DVE rate)
                                nc.vector.tensor_mul(e_sb[:], e_sb[:], mask_sb[:])
                            if pend is not None:
                                _emit_pv(nc, o_ab, v_sb, *pend)
                            pend = (e_sb, g, gi == 0, gi == J)
                        _emit_pv(nc, o_ab, v_sb, *pend)

                        # normalize: 1/Z from o_ab row 64, replicated across
                        # partitions on the Pool engine, multiplied in as the
                        # attention output is copied to SBUF (bf16)
                        rz = rzpool.tile([1, 512], F32)
                        nc.vector.reciprocal(rz[:], o_ab[64:65, :])
                        bc = bcpool.tile([64, 512], F32)
                        nc.gpsimd.partition_broadcast(bc[:], rz[0:1, :])
                        nc.vector.tensor_mul(
                            aoT[0:64, qsl], o_ab[0:64, 0:256], bc[:, 0:256]
                        )
                        nc.vector.tensor_mul(
                            stg[0:64, qsl], o_ab[0:64, 256:512], bc[:, 256:512]
                        )
                        # restack slot-b rows into partitions 64..127
                        nc.sync.dma_start(out=aoT[64:128, qsl], in_=stg[0:64, qsl])
                    # o-projection deferred one J so the PE never waits on
                    # the restack DMA chain
                    if J > 0:
                        emit_oproj(J - 1)
                emit_oproj(NJ - 1)

    nc.compile()
    return nc


def _emit_pv(nc, o_ab, v_sb, e_sb, g, first, last):
    """PV accumulation for one exp'd group (k-blocks 2g, 2g+1)."""
    for i, kb in enumerate((2 * g, 2 * g + 1)):
        for sl in range(2):
            seg = sl * 512 + i * 256
            # one accumulation group for the whole o_ab tile: start=True
            # clears has_written for the entire PSUM bank, so only the very
            # first matmul may set it
            nc.tensor.matmul(
                o_ab[:, sl * 256 : (sl + 1) * 256],
                v_sb[:, kb * 65 : (kb + 1) * 65],
                e_sb[:, seg : seg + 256],
                start=(first and i == 0 and sl == 0),
                stop=(last and i == 1 and sl == 1),
                skip_group_check=True,
            )


def _rope_tables():
    """[64, S] cos and pre-swap sign-folded sin: rope(x) = x*cos +
    perm(x*sind) with perm the half-swap, so sind rows 32:64 carry the
    minus sign (they land in rows 0:32 after the swap)."""
    inv_freq = 1.0 / (
        ROPE_THETA ** (np.arange(0, HEAD_DIM, 2, dtype=np.float32) / HEAD_DIM)
    )
    t = np.arange(S, dtype=np.float32)
    freqs = np.outer(t, inv_freq)  # [S, 32]
    emb = np.concatenate([freqs, freqs], axis=-1)  # [S, 64]
    cosd = np.cos(emb).T.astype(np.float32)  # [64, S]
    sind = np.sin(emb).T.astype(np.float32)
    sind[32:64] = -sind[32:64]
    return np.ascontiguousarray(cosd.astype(BF)), np.ascontiguousarray(
        sind.astype(BF)
    )


def _perms():
    """[128, 384] bf16: permQ | permKs | permKc (matmul lhsT layout:
    lhsT[p, i] = 1 selects input partition p for output partition i)."""
    p = np.zeros((128, 384), np.float32)
    for i in range(128):
        blk, d = (i // 64) * 64, i % 64
        p[blk + (d + 32) % 64, i] = 1.0          # permQ: half-swap per slot
        p[(d + 32) % 64, 128 + i] = 1.0          # permKs: swap + duplicate
        p[d, 256 + i] = 1.0                      # permKc: duplicate
    return np.ascontiguousarray(p.astype(BF))


def _masks():
    kp = np.arange(128)[:, None]
    qp = np.arange(128)[None, :]
    tri = np.where(kp <= qp, 1.0, 0.0).astype(np.float32)  # [128,128]
    ones = np.ones((128, 128), np.float32)
    zeros = np.zeros((128, 128), np.float32)
    mask0 = np.concatenate([tri, ones], axis=1)   # kb 2J vs [2J, 2J+1]
    mask1 = np.concatenate([zeros, tri], axis=1)  # kb 2J+1 vs [2J, 2J+1]
    return np.ascontiguousarray(
        np.concatenate([mask0, mask1, mask0, mask1], axis=1).astype(BF)
    )  # [128, 1024]


def _tile_hsT(hsT):
    """[896, 2048] -> [512, 3584]: row ss*128+p = concat over t of
    hsT[t*128+p, ss*512:(ss+1)*512], matching the SBUF projection layout."""
    out = np.empty((4 * 128, NH7 * 512), BF)
    for ss in range(4):
        blk = hsT[:, ss * 512 : (ss + 1) * 512].reshape(NH7, 128, 512)
        out[ss * 128 : (ss + 1) * 128, :] = (
            blk.transpose(1, 0, 2).reshape(128, NH7 * 512).astype(BF)
        )
    return np.ascontiguousarray(out)


_CONST_CACHE = None


def make_in_maps(hidden_states, wq, bq, wk, bk, wv, bv, wo):
    global _CONST_CACHE
    if _CONST_CACHE is None:
        cosd, sind = _rope_tables()
        _CONST_CACHE = (cosd, sind, _masks(), _perms())
    cosd, sind, maskD, permD = _CONST_CACHE
    # the tiled hidden states are shared by the 4 cores of a batch
    hs_tiled = [_tile_hsT(hidden_states[b].T) for b in range(B)]
    in_maps = []
    for core in range(8):
        b, kv, half = core // 4, (core % 4) // 2, core % 2
        if half == 0:
            slots = [kv * 7 + 0, kv * 7 + 1, kv * 7 + 2, kv * 7 + 3]
            dup = []
        else:
            slots = [kv * 7 + 4, kv * 7 + 5, kv * 7 + 6, kv * 7 + 3]
            dup = [3]
        cols = np.concatenate([np.arange(h * 64, (h + 1) * 64) for h in slots])
        wq4 = np.ascontiguousarray(wq[:, cols].astype(BF))
        bq4 = np.ascontiguousarray(bq[cols].reshape(2, 128))
        wkv = np.ascontiguousarray(
            np.concatenate(
                [wk[:, kv * 64 : (kv + 1) * 64], wv[:, kv * 64 : (kv + 1) * 64]],
                axis=1,
            ).astype(BF)
        )
        bkv = np.ascontiguousarray(
            np.concatenate(
                [bk[kv * 64 : (kv + 1) * 64], bv[kv * 64 : (kv + 1) * 64]]
            ).reshape(1, 128)
        )
        wo4 = wo[cols, :].copy()
        for d in dup:
            wo4[d * 64 : (d + 1) * 64, :] = 0.0
        in_maps.append(
            {
                "hsT": hs_tiled[b],
                "wq4": wq4,
                "bq4": bq4,
                "wkv": wkv,
                "bkv": bkv,
                "wo4": np.ascontiguousarray(wo4.astype(BF)),
                "cosd": cosd,
                "sind": sind,
                "maskD": maskD,
                "permD": permD,
            }
        )
    return in_maps


_NC_CACHE = None


def _get_program():
    global _NC_CACHE
    if _NC_CACHE is None:
        _NC_CACHE = build_program()
    return _NC_CACHE


def kernel(hidden_states, wq, bq, wk, bk, wv, bv, wo):
    hidden_states = np.asarray(hidden_states, np.float32)
    wq = np.asarray(wq, np.float32)
    bq = np.asarray(bq, np.float32)
    wk = np.asarray(wk, np.float32)
    bk = np.asarray(bk, np.float32)
    wv = np.asarray(wv, np.float32)
    bv = np.asarray(bv, np.float32)
    wo = np.asarray(wo, np.float32)

    nc = _get_program()
    in_maps = make_in_maps(hidden_states, wq, bq, wk, bk, wv, bv, wo)
    res = run_bass_kernel_spmd(nc, in_maps, list(range(8)))
    out = np.zeros((B, S, HIDDEN), np.float32)
    for core in range(8):
        out[core // 4] += res.results[core]["out"]
    return out


# revision 29
# speedup vs baseline: 1.1280x; 1.1071x over previous
"""Trainium2 Bass kernel for GQA attention (nn_Attention_40364102648437).

Problem: B=2, S=2048, HIDDEN=896, 14 q heads / 2 kv heads, head_dim 64,
RoPE (theta 1e6), causal softmax, o-projection.

Sharding (8 cores, SPMD): core = b*4 + kv*2 + half.
Each core owns one batch b, one kv head, and 4 q-head slots (7 q heads per
kv group are split 4+3; the last slot of the second half is a duplicate
whose wo rows are zeroed so its contribution vanishes). Every core computes
a full [S, HIDDEN] partial output (its heads' contribution through wo);
the host sums the 4 partials per batch.

Engine budget (cost model): PE does all matmuls (~92us at 2.4GHz), the
Activation engine does ONLY the softmax exp (72 x [128,1024] ~= 75us), DVE
does RoPE/mask/bias/normalize element-wise work in bf16 (2-byte dtypes get
2-4x DVE rate), Pool (gpsimd) does PSUM->SBUF copies and the softmax
1/Z partition-broadcast, SP issues every DMA. The whole attention path is
bf16 (same PE rate as fp32r, half the DMA bytes, no fp32r-producer rule).

Softmax normalization: V tiles carry a ones column so PV accumulates the
denominator Z in o_ab row 64; 1/Z is computed by DVE reciprocal into a
[1,512] SBUF row, replicated across partitions with gpsimd
partition_broadcast (no DRAM bounce), and multiplied into the attention
output as it is copied to SBUF (bf16) for the o-projection.

Pipelining: PV trails scores/exp by one k-group; the o-projection of
superblock J is emitted after attention of J+1's first pair so the PE
never waits on the slot-b restack DMA; output rows DMA straight from an
SBUF staging tile.

Hardware constraints (from the previous session, kept intact):
  - concurrent row-group matmuls (partition bases 0/64) must write
    different PSUM banks -> s_ps puts slot a in bank 0, slot b in bank 1;
  - matmul start=True clears has_written for its PSUM region, so o_ab
    gets exactly one start/stop accumulation group;
  - engines cannot move data across partitions: RoPE's rotate-half swap,
    the k row duplication, and the slot-b restack use SBUF->SBUF DMA;
  - tensor_tensor may read only one input from PSUM.
"""

import numpy as np
import ml_dtypes

import concourse.bass as bass
import concourse.mybir as mybir
from concourse import bacc
from concourse.tile import TileContext
from concourse.masks import make_identity
from concourse.bass_utils import run_bass_kernel_spmd

F32 = mybir.dt.float32
BF16 = mybir.dt.bfloat16
BF = ml_dtypes.bfloat16

HIDDEN = 896
HEAD_DIM = 64
B = 2
S = 2048
ROPE_THETA = 1000000.0
NH7 = HIDDEN // 128  # 7 hidden tiles
NKB = S // 128       # 16 key blocks
NJ = S // 256        # 8 query superblocks (256 q positions each)


def build_program():
    nc = bacc.Bacc("TRN2", target_bir_lowering=False, debug=False, num_devices=8)

    # host-pre-tiled: row ss*128+p holds [t, n] -> hs[b][ss*512+n, t*128+p]
    hsT = nc.dram_tensor("hsT", [4 * 128, NH7 * 512], BF16, kind="ExternalInput")
    wq4 = nc.dram_tensor("wq4", [HIDDEN, 256], BF16, kind="ExternalInput")
    bq4 = nc.dram_tensor("bq4", [2, 128], F32, kind="ExternalInput")
    wkv = nc.dram_tensor("wkv", [HIDDEN, 128], BF16, kind="ExternalInput")
    bkv = nc.dram_tensor("bkv", [1, 128], F32, kind="ExternalInput")
    wo4 = nc.dram_tensor("wo4", [256, HIDDEN], BF16, kind="ExternalInput")
    cosd = nc.dram_tensor("cosd", [128, S], BF16, kind="ExternalInput")
    sind = nc.dram_tensor("sind", [128, S], BF16, kind="ExternalInput")
    maskD = nc.dram_tensor("maskD", [128, 1024], BF16, kind="ExternalInput")
    # rotate-half permutations as matmul weights: permQ does the half-swap
    # within each 64-row slot; permKs/permKc (cols 128:256 / 256:384, rows
    # 0:64) swap and duplicate k into both partition halves
    permD = nc.dram_tensor("permD", [128, 384], BF16, kind="ExternalInput")
    out_d = nc.dram_tensor("out", [S, HIDDEN], F32, kind="ExternalOutput")

    EXP = mybir.ActivationFunctionType.Exp

    with TileContext(nc) as tc:
        with (
            tc.tile_pool(name="const", bufs=1) as cpool,
            tc.tile_pool(name="big", bufs=1) as bigpool,
        ):
            # ---- constants, issued in first-use order (DMAs serialize on
            # the single HWDGE device at ~625ns each)
            wkv_sb = cpool.tile([128, NH7 * 128], BF16)
            nc.sync.dma_start(
                out=wkv_sb[:].rearrange("p (t f) -> p t f", t=NH7),
                in_=wkv.rearrange("(t p) f -> p t f", p=128),
            )
            bkv_sb = cpool.tile([128, 1], F32)
            nc.sync.dma_start(out=bkv_sb[:], in_=bkv.rearrange("a p -> p a"))
            perm_sb = cpool.tile([128, 384], BF16)
            nc.sync.dma_start(out=perm_sb[:], in_=permD[:])
            wq_sb = cpool.tile([128, NH7 * 256], BF16)
            bq_sb = cpool.tile([128, 2], F32)
            cos_sb = cpool.tile([128, S], BF16)
            sin_sb = cpool.tile([128, S], BF16)
            wo_sb = cpool.tile([128, 2 * HIDDEN], BF16)
            woB_sb = cpool.tile([64, 2 * HIDDEN], BF16)
            mask_sb = cpool.tile([128, 1024], BF16)
            identb = cpool.tile([128, 128], BF16)
            make_identity(nc, identb[:])
            # force the Exp activation table load off the critical path
            warm = cpool.tile([1, 8], F32)
            nc.vector.memset(warm[:], 0.0)
            nc.scalar.activation(
                warm[:], warm[:], mybir.ActivationFunctionType.Exp,
                bias=0.0, scale=1.0,
            )

            def load_consts_pre():
                nc.sync.dma_start(
                    out=wq_sb[:].rearrange("p (t f) -> p t f", t=NH7),
                    in_=wq4.rearrange("(t p) f -> p t f", p=128),
                )
                nc.sync.dma_start(out=bq_sb[:], in_=bq4.rearrange("a p -> p a"))

            def load_consts_mid():
                nc.sync.dma_start(out=cos_sb[:], in_=cosd[:])
                nc.sync.dma_start(out=sin_sb[:], in_=sind[:])
                nc.sync.dma_start(out=mask_sb[:], in_=maskD[:])

            def load_consts_post():
                nc.sync.dma_start(
                    out=wo_sb[:].rearrange("p (t f) -> p t f", t=2),
                    in_=wo4.rearrange("(t p) f -> p t f", p=128),
                )
                nc.sync.dma_start(
                    out=woB_sb[:].rearrange("p (t f) -> p t f", t=2),
                    in_=wo4.rearrange("(t p) f -> p t f", p=128)[64:128],
                )

            # ---- persistent activations (all bf16)
            kvT = bigpool.tile([128, S], BF16)   # rows 0-63 k, 64-127 vT
            kdr = bigpool.tile([128, S], BF16)   # rope'd k, duplicated halves
            qA = bigpool.tile([128, S], BF16)
            qB = bigpool.tile([128, S], BF16)
            qAr = bigpool.tile([128, S], BF16)
            qBr = bigpool.tile([128, S], BF16)
            v_sb = bigpool.tile([128, NKB * 65], BF16)
            aoT0 = bigpool.tile([128, S], BF16)
            aoT1 = bigpool.tile([128, S], BF16)
            stg0 = bigpool.tile([64, S], BF16)
            stg1 = bigpool.tile([64, S], BF16)

            # ================= phase A: projections =================
            # Interleaved per hidden-states chunk: kv-proj, q-proj, then
            # (after each odd ss) RoPE + V transposes for that 1024-column
            # chunk, so attention starts as soon as chunk 0 is rope'd
            # instead of after the whole prologue.
            with (
                tc.tile_pool(name="hst", bufs=4) as hpool,
                tc.tile_pool(name="pps", bufs=3, space="PSUM") as ppool,
                tc.tile_pool(name="swp", bufs=3) as swpool,
                tc.tile_pool(name="vtr", bufs=2, space="PSUM") as vpool,
            ):
                def rope_chunk(c):
                    csl = slice(c * 1024, (c + 1) * 1024)
                    # k: multiply-then-permute; permKs/permKc also duplicate
                    # the rope'd k into both partition halves
                    ks = swpool.tile([64, 1024], BF16, tag="ks")
                    nc.vector.tensor_mul(ks[:], kvT[0:64, csl], sin_sb[0:64, csl])
                    nc.vector.tensor_mul(
                        kvT[0:64, csl], kvT[0:64, csl], cos_sb[0:64, csl]
                    )
                    for h in range(2):
                        hsl = slice(c * 1024 + h * 512, c * 1024 + (h + 1) * 512)
                        lsl = slice(h * 512, (h + 1) * 512)
                        r_ps = ppool.tile([128, 512], F32, tag="pps")
                        nc.tensor.matmul(
                            r_ps[:], perm_sb[0:64, 128:256], ks[:, lsl],
                            start=True, stop=False,
                        )
                        nc.tensor.matmul(
                            r_ps[:], perm_sb[0:64, 256:384], kvT[0:64, hsl],
                            start=False, stop=True,
                        )
                        nc.gpsimd.tensor_copy(kdr[:, hsl], r_ps[:])
                    # v for this chunk's k-blocks
                    for kb in range(c * 8, c * 8 + 8):
                        vt_ps = vpool.tile([128, 64], BF16)
                        nc.tensor.transpose(
                            vt_ps[:],
                            kvT[64:128, kb * 128 : (kb + 1) * 128],
                            identb[64:128, 64:128],
                        )
                        nc.gpsimd.tensor_copy(
                            v_sb[:, kb * 65 : kb * 65 + 64], vt_ps[:]
                        )
                    nc.gpsimd.memset(
                        v_sb[:, c * 8 * 65 + 64 : (c + 1) * 8 * 65 : 65], 1.0
                    )
                    # q tensors
                    for t, tr in ((qA, qAr), (qB, qBr)):
                        qs = swpool.tile([128, 1024], BF16, tag="qs")
                        nc.vector.tensor_mul(qs[:], t[:, csl], sin_sb[:, csl])
                        nc.vector.tensor_mul(t[:, csl], t[:, csl], cos_sb[:, csl])
                        for h in range(2):
                            hsl = slice(
                                c * 1024 + h * 512, c * 1024 + (h + 1) * 512
                            )
                            lsl = slice(h * 512, (h + 1) * 512)
                            r_ps = ppool.tile([128, 512], F32, tag="pps")
                            nc.tensor.matmul(
                                r_ps[:], perm_sb[:, 0:128], qs[:, lsl],
                                start=True, stop=False,
                            )
                            nc.tensor.matmul(
                                r_ps[:], identb[:], t[:, hsl],
                                start=False, stop=True,
                            )
                            eng = nc.gpsimd if h == 0 else nc.vector
                            eng.tensor_copy(tr[:, hsl], r_ps[:])

                hs_tiles = []
                for ss in range(4):
                    ssl = slice(ss * 512, (ss + 1) * 512)
                    hs_t = hpool.tile([128, NH7 * 512], BF16)
                    hs_tiles.append(hs_t)
                    if ss == 0:
                        # split so the first kv matmuls start sooner
                        nc.sync.dma_start(
                            out=hs_t[:, 0 : 3 * 512], in_=hsT[0:128, 0 : 3 * 512]
                        )
                        nc.sync.dma_start(
                            out=hs_t[:, 3 * 512 :], in_=hsT[0:128, 3 * 512 :]
                        )
                        load_consts_pre()
                    else:
                        nc.sync.dma_start(
                            out=hs_t[:], in_=hsT[ss * 128 : (ss + 1) * 128, :]
                        )
                    if ss == 1:
                        load_consts_mid()
                    if ss == 3:
                        load_consts_post()
                    kv_ps = ppool.tile([128, 512], F32, tag="pps")
                    for h in range(NH7):
                        nc.tensor.matmul(
                            kv_ps[:],
                            wkv_sb[:, h * 128 : (h + 1) * 128],
                            hs_t[:, h * 512 : (h + 1) * 512],
                            start=(h == 0),
                            stop=(h == NH7 - 1),
                        )
                    nc.vector.tensor_scalar_add(kvT[:, ssl], kv_ps[:], bkv_sb[:, 0:1])
                    for ft in range(2):
                        q_ps = ppool.tile([128, 512], F32, tag="pps")
                        for h in range(NH7):
                            nc.tensor.matmul(
                                q_ps[:],
                                wq_sb[:, h * 256 + ft * 128 : h * 256 + (ft + 1) * 128],
                                hs_t[:, h * 512 : (h + 1) * 512],
                                start=(h == 0),
                                stop=(h == NH7 - 1),
                            )
                        qt = (qA, qB)[ft]
                        nc.vector.tensor_scalar_add(
                            qt[:, ssl], q_ps[:], bq_sb[:, ft : ft + 1]
                        )
                    if ss % 2 == 1:
                        rope_chunk(ss // 2)

            # ================= phase B: attention + o-projection =================
            with (
                tc.tile_pool(name="sps", bufs=2, space="PSUM") as spool,
                tc.tile_pool(name="ops", bufs=2, space="PSUM") as opool,
                tc.tile_pool(name="fps", bufs=2, space="PSUM") as fpool,
                tc.tile_pool(name="esb", bufs=4) as epool,
                tc.tile_pool(name="rzs", bufs=2) as rzpool,
                tc.tile_pool(name="bcs", bufs=2) as bcpool,
                tc.tile_pool(name="osb", bufs=3) as obpool,
            ):
                def emit_oproj(J, from_stg=False):
                    for qb in (2 * J, 2 * J + 1):
                        ob = obpool.tile([128, HIDDEN], F32)
                        for half in range(2):
                            hsl = slice(half * 448, (half + 1) * 448)
                            f_ps = fpool.tile([128, 448], F32)
                            for ft in range(2):
                                aoTt = (aoT0, aoT1)[ft]
                                wsl = slice(ft * HIDDEN + half * 448,
                                            ft * HIDDEN + (half + 1) * 448)
                                if from_stg:
                                    # last J: slot-b rows straight from the
                                    # staging tile, skipping the restack DMA
                                    nc.tensor.matmul(
                                        f_ps[:],
                                        aoTt[0:64, qb * 128 : (qb + 1) * 128],
                                        wo_sb[0:64, wsl],
                                        start=(ft == 0),
                                        stop=False,
                                    )
                                    nc.tensor.matmul(
                                        f_ps[:],
                                        (stg0, stg1)[ft][:, qb * 128 : (qb + 1) * 128],
                                        woB_sb[:, wsl],
                                        start=False,
                                        stop=(ft == 1),
                                    )
                                else:
                                    nc.tensor.matmul(
                                        f_ps[:],
                                        aoTt[:, qb * 128 : (qb + 1) * 128],
                                        wo_sb[:, wsl],
                                        start=(ft == 0),
                                        stop=(ft == 1),
                                    )
                            nc.gpsimd.tensor_copy(ob[:, hsl], f_ps[:])
                        nc.sync.dma_start(
                            out=out_d[qb * 128 : (qb + 1) * 128, :], in_=ob[:]
                        )

                for J in range(NJ):
                    for pair in range(2):
                        qt = (qAr, qBr)[pair]
                        aoT = (aoT0, aoT1)[pair]
                        stg = (stg0, stg1)[pair]
                        qsl = slice(J * 256, (J + 1) * 256)
                        o_ab = opool.tile([65, 512], F32)
                        pend = None  # software pipeline: PV trails S^T/exp by 1
                        # diagonal group first: its exp -> mask -> PV chain
                        # pipelines like any other group instead of stalling
                        # the PE at the end of the pair
                        gorder = [J] + list(range(J))
                        for gi, g in enumerate(gorder):
                            s_ps = spool.tile([128, 1024], F32)
                            for i, kb in enumerate((2 * g, 2 * g + 1)):
                                for half in range(2):
                                    # concurrent row-group pair must write
                                    # different PSUM banks: slot a bank 0,
                                    # slot b bank 1
                                    seg = half * 512 + i * 256
                                    nc.tensor.matmul(
                                        s_ps[:, seg : seg + 256],
                                        kdr[half * 64 : (half + 1) * 64,
                                            kb * 128 : (kb + 1) * 128],
                                        qt[half * 64 : (half + 1) * 64, qsl],
                                        start=True,
                                        stop=True,
                                    )
                            e_sb = epool.tile([128, 1024], BF16)
                            nc.scalar.activation(
                                e_sb[:], s_ps[:], EXP, bias=0.0, scale=0.125
                            )
                            if g == J:
                                # multiplicative 0/1 causal mask after exp
                                # (bf16, all-SBUF -> 4x DVE rate)
                                nc.vector.tensor_mul(e_sb[:], e_sb[:], mask_sb[:])
                            if pend is not None:
                                _emit_pv(nc, o_ab, v_sb, *pend)
                            pend = (e_sb, g, gi == 0, gi == J)
                        _emit_pv(nc, o_ab, v_sb, *pend)

                        # normalize: 1/Z from o_ab row 64, replicated across
                        # partitions on the Pool engine, multiplied in as the
                        # attention output is copied to SBUF (bf16)
                        rz = rzpool.tile([1, 512], F32)
                        nc.vector.reciprocal(rz[:], o_ab[64:65, :])
                        bc = bcpool.tile([64, 512], F32)
                        nc.gpsimd.partition_broadcast(bc[:], rz[0:1, :])
                        nc.vector.tensor_mul(
                            aoT[0:64, qsl], o_ab[0:64, 0:256], bc[:, 0:256]
                        )
                        nc.vector.tensor_mul(
                            stg[0:64, qsl], o_ab[0:64, 256:512], bc[:, 256:512]
                        )
                        if J < NJ - 1:
                            # restack slot-b rows into partitions 64..127
                            nc.sync.dma_start(
                                out=aoT[64:128, qsl], in_=stg[0:64, qsl]
                            )
                    # o-projection deferred one J so the PE never waits on
                    # the restack DMA chain
                    if J > 0:
                        emit_oproj(J - 1)
                emit_oproj(NJ - 1, from_stg=True)

    nc.compile()
    return nc


def _emit_pv(nc, o_ab, v_sb, e_sb, g, first, last):
    """PV accumulation for one exp'd group (k-blocks 2g, 2g+1)."""
    for i, kb in enumerate((2 * g, 2 * g + 1)):
        for sl in range(2):
            seg = sl * 512 + i * 256
            # one accumulation group for the whole o_ab tile: start=True
            # clears has_written for the entire PSUM bank, so only the very
            # first matmul may set it
            nc.tensor.matmul(
                o_ab[:, sl * 256 : (sl + 1) * 256],
                v_sb[:, kb * 65 : (kb + 1) * 65],
                e_sb[:, seg : seg + 256],
                start=(first and i == 0 and sl == 0),
                stop=(last and i == 1 and sl == 1),
                skip_group_check=True,
            )


def _rope_tables():
    """[64, S] cos and pre-swap sign-folded sin: rope(x) = x*cos +
    perm(x*sind) with perm the half-swap, so sind rows 32:64 carry the
    minus sign (they land in rows 0:32 after the swap)."""
    inv_freq = 1.0 / (
        ROPE_THETA ** (np.arange(0, HEAD_DIM, 2, dtype=np.float32) / HEAD_DIM)
    )
    t = np.arange(S, dtype=np.float32)
    freqs = np.outer(t, inv_freq)  # [S, 32]
    emb = np.concatenate([freqs, freqs], axis=-1)  # [S, 64]
    cosd = np.cos(emb).T.astype(np.float32)  # [64, S]
    sind = np.sin(emb).T.astype(np.float32)
    sind[32:64] = -sind[32:64]
    cosd = np.concatenate([cosd, cosd], axis=0)  # [128, S]
    sind = np.concatenate([sind, sind], axis=0)
    return np.ascontiguousarray(cosd.astype(BF)), np.ascontiguousarray(
        sind.astype(BF)
    )


def _perms():
    """[128, 384] bf16: permQ | permKs | permKc (matmul lhsT layout:
    lhsT[p, i] = 1 selects input partition p for output partition i)."""
    p = np.zeros((128, 384), np.float32)
    for i in range(128):
        blk, d = (i // 64) * 64, i % 64
        p[blk + (d + 32) % 64, i] = 1.0          # permQ: half-swap per slot
        p[(d + 32) % 64, 128 + i] = 1.0          # permKs: swap + duplicate
        p[d, 256 + i] = 1.0                      # permKc: duplicate
    return np.ascontiguousarray(p.astype(BF))


def _masks():
    kp = np.arange(128)[:, None]
    qp = np.arange(128)[None, :]
    tri = np.where(kp <= qp, 1.0, 0.0).astype(np.float32)  # [128,128]
    ones = np.ones((128, 128), np.float32)
    zeros = np.zeros((128, 128), np.float32)
    mask0 = np.concatenate([tri, ones], axis=1)   # kb 2J vs [2J, 2J+1]
    mask1 = np.concatenate([zeros, tri], axis=1)  # kb 2J+1 vs [2J, 2J+1]
    return np.ascontiguousarray(
        np.concatenate([mask0, mask1, mask0, mask1], axis=1).astype(BF)
    )  # [128, 1024]


def _tile_hsT(hsT):
    """[896, 2048] -> [512, 3584]: row ss*128+p = concat over t of
    hsT[t*128+p, ss*512:(ss+1)*512], matching the SBUF projection layout."""
    out = np.empty((4 * 128, NH7 * 512), BF)
    for ss in range(4):
        blk = hsT[:, ss * 512 : (ss + 1) * 512].reshape(NH7, 128, 512)
        out[ss * 128 : (ss + 1) * 128, :] = (
            blk.transpose(1, 0, 2).reshape(128, NH7 * 512).astype(BF)
        )
    return np.ascontiguousarray(out)


_CONST_CACHE = None


def make_in_maps(hidden_states, wq, bq, wk, bk, wv, bv, wo):
    global _CONST_CACHE
    if _CONST_CACHE is None:
        cosd, sind = _rope_tables()
        _CONST_CACHE = (cosd, sind, _masks(), _perms())
    cosd, sind, maskD, permD = _CONST_CACHE
    # the tiled hidden states are shared by the 4 cores of a batch
    hs_tiled = [_tile_hsT(hidden_states[b].T) for b in range(B)]
    in_maps = []
    for core in range(8):
        b, kv, half = core // 4, (core % 4) // 2, core % 2
        if half == 0:
            slots = [kv * 7 + 0, kv * 7 + 1, kv * 7 + 2, kv * 7 + 3]
            dup = []
        else:
            slots = [kv * 7 + 4, kv * 7 + 5, kv * 7 + 6, kv * 7 + 3]
            dup = [3]
        cols = np.concatenate([np.arange(h * 64, (h + 1) * 64) for h in slots])
        wq4 = np.ascontiguousarray(wq[:, cols].astype(BF))
        bq4 = np.ascontiguousarray(bq[cols].reshape(2, 128))
        wkv = np.ascontiguousarray(
            np.concatenate(
                [wk[:, kv * 64 : (kv + 1) * 64], wv[:, kv * 64 : (kv + 1) * 64]],
                axis=1,
            ).astype(BF)
        )
        bkv = np.ascontiguousarray(
            np.concatenate(
                [bk[kv * 64 : (kv + 1) * 64], bv[kv * 64 : (kv + 1) * 64]]
            ).reshape(1, 128)
        )
        wo4 = wo[cols, :].copy()
        for d in dup:
            wo4[d * 64 : (d + 1) * 64, :] = 0.0
        in_maps.append(
            {
                "hsT": hs_tiled[b],
                "wq4": wq4,
                "bq4": bq4,
                "wkv": wkv,
                "bkv": bkv,
                "wo4": np.ascontiguousarray(wo4.astype(BF)),
                "cosd": cosd,
                "sind": sind,
                "maskD": maskD,
                "permD": permD,
            }
        )
    return in_maps


_NC_CACHE = None


def _get_program():
    global _NC_CACHE
    if _NC_CACHE is None:
        _NC_CACHE = build_program()
    return _NC_CACHE


def kernel(hidden_states, wq, bq, wk, bk, wv, bv, wo):
    hidden_states = np.asarray(hidden_states, np.float32)
    wq = np.asarray(wq, np.float32)
    bq = np.asarray(bq, np.float32)
    wk = np.asarray(wk, np.float32)
    bk = np.asarray(bk, np.float32)
    wv = np.asarray(wv, np.float32)
    bv = np.asarray(bv, np.float32)
    wo = np.asarray(wo, np.float32)

    nc = _get_program()
    in_maps = make_in_maps(hidden_states, wq, bq, wk, bk, wv, bv, wo)
    res = run_bass_kernel_spmd(nc, in_maps, list(range(8)))
    out = np.zeros((B, S, HIDDEN), np.float32)
    for core in range(8):
        out[core // 4] += res.results[core]["out"]
    return out
